# revision 21
# baseline (speedup 1.0000x reference)
"""BinaryLinear TRN2 kernel: z = x @ sign(weight).T + bias.

x [8192, 4096] f32, weight [4096, 4096] f32, bias [4096] f32 (zeros).

Strategy (8 NeuronCores, SPMD, data-parallel over the 8192-token batch;
weight replicated; host does layout/transport prep only, all arithmetic
on device):

  - bf16 single-pass matmul. The PE moving feed is the limit: bf16
    streams 1 column/cycle (16384 MAC/cyc); fp8 DoubleRow doubles that
    but needs a two-pass hi/lo split for precision, landing at the same
    ~437 us/core floor (measured 444-454 us on HW for both). bf16
    single-pass hits the floor with half the instructions, no x
    quantization, FWL-eligible 128-col weight loads, ~1.6e-3 rel err.
  - Host ships bf16(2*x_shard.T) and fp8(w.T) (sign-exact transport,
    half the bytes) in BLOCK-MAJOR layouts so every DMA is long
    contiguous runs per partition (2-8 KB) - transposed-layout 512 B
    runs were the previous bottleneck (~90 GB/s effective; the 33.5 MB
    weight stream dominated the kernel).
  - Device binarize: bitcast the fp8 weight to uint8 and take the sign
    bit: (u8 is_lt 128) - 0.5 = sign(w)/2 with sign(-0)=-1 for
    rounded-up tiny negatives and sign(+0)=+1 - exactly matching the
    reference's sign(0)=+1. One DVE/Pool tensor_scalar per sub-chunk.
    z = (sign(w)/2) @ (2x) exactly.
  - PE runs ONLY matmuls: stationary [128,128] bf16 +-0.5 weights,
    moving 512 tokens, psum [128,512] (one bank), 32-deep accumulation,
    stationary shared across the 2 token chunks (redundant LDWEIGHTS
    deleted post-legalization - the PE array keeps its weights).
  - x streams on the SP DMA queue (prioritized at startup), w blocks on
    the ACT queue in 8-plane sub-chunks binarized ahead (2 blocks in
    flight); z written as bf16 zT, host transposes/upcasts on gather.
"""

import numpy as np
import ml_dtypes

import concourse.bacc as bacc
import concourse.bass as bass
import concourse.mybir as mybir
import concourse.tile as tile
from concourse import bass_utils
from concourse.bass import ts

P = 128
N_CORES = 8
N_TOK, K_IN, N_OUT = 8192, 4096, 4096
T = N_TOK // N_CORES  # 1024 tokens per core
KT = K_IN // P  # 32 k-planes
XG = 4  # x planes per DMA group
NXG = KT // XG  # 8 x tiles
OB = 256  # out-feature block width
NOB = N_OUT // OB  # 16 blocks
TH = 512  # token chunk per psum tile
NTH = T // TH  # 2
WCH = 8  # k-planes per w prep sub-chunk
NWCH = KT // WCH  # 4 sub-chunks per block

F32 = mybir.dt.float32
BF16 = mybir.dt.bfloat16
F8 = mybir.dt.float8e4
U8 = mybir.dt.uint8

_cached_nc = None


def _dedupe_ldweights(nc):
    """Remove InstLdweights whose weight AP is identical to the previous
    one with only InstMatmult instructions in between (the PE array still
    holds those weights). Waits on a removed load are hoisted onto the
    next PE instruction."""
    n_removed = 0
    for blk in nc.m.functions[0].blocks:
        insts = blk.instructions
        keep = []
        last_ld_key = None
        pending_waits = []
        for inst in insts:
            if inst.engine != mybir.EngineType.PE:
                keep.append(inst)
                continue
            if isinstance(inst, mybir.InstLdweights):
                key = str(inst.ins[0]) + f"|{inst.perf_mode}"
                if key == last_ld_key:
                    si = inst.sync_info
                    if si is not None:
                        pending_waits.extend(si.on_wait)
                        assert not si.on_update, "dedupe: LD carries updates"
                    n_removed += 1
                    continue
                last_ld_key = key
            elif not isinstance(inst, mybir.InstMatmult):
                last_ld_key = None
            if pending_waits:
                si = inst.sync_info
                if si is None:
                    inst.sync_info = mybir.SyncInfo(
                        on_wait=pending_waits, on_update=[]
                    )
                else:
                    si.on_wait = list(si.on_wait) + pending_waits
                pending_waits = []
            keep.append(inst)
        assert not pending_waits
        blk.instructions = keep
    return n_removed


def _build_program(loop: int = 0):
    """loop=0: plain kernel. loop=L>0: body wrapped in an on-device For_i
    (used for HW timing via the slope method)."""
    nc = bacc.Bacc("TRN2", target_bir_lowering=False, debug=False)
    # bf16(2*x) shard: [group, kp, plane-in-group, tok] (8 KB runs)
    x2_d = nc.dram_tensor("x2", [NXG, P, XG, T], BF16, kind="ExternalInput")
    # fp8(w.T), block-major: [block, kp, plane, out-in-block] (8 KB runs)
    w_d = nc.dram_tensor("wt", [NOB, P, KT, OB], F8, kind="ExternalInput")
    zs_d = nc.dram_tensor("zs", [N_OUT, T], F32, kind="ExternalOutput")

    import contextlib

    IS_LT = mybir.AluOpType.is_lt
    SUB = mybir.AluOpType.subtract

    with tile.TileContext(nc) as tc:
        with (
            tc.tile_pool(name="xt", bufs=1) as xtp,
            tc.tile_pool(name="wraw", bufs=8) as wrawp,
            tc.tile_pool(name="wbin", bufs=3) as wbinp,
            tc.tile_pool(name="zst", bufs=3) as zstp,
            tc.tile_pool(name="psm", bufs=6, space="PSUM") as psmp,
        ):
            # resident bf16 x tiles, one per XG-plane group
            xt = [
                xtp.tile([P, XG, T], BF16, name=f"xt{i}") for i in range(NXG)
            ]

            loop_cm = tc.For_i(0, loop, 1) if loop else contextlib.nullcontext()
            with loop_cm:
                # ---- Weight block prep (ACT hwdge queue): DMA fp8 wT
                # sub-chunks (contiguous 2 KB runs), binarize via the uint8
                # sign bit to +-0.5 bf16 on DVE/Pool ----
                def prep(ob, eng_off=0):
                    subs = []
                    for s in range(NWCH):
                        wr = wrawp.tile([P, WCH, OB], F8, name="wr", tag="wr")
                        nc.scalar.dma_start(
                            wr[:], w_d.ap()[ob, :, s * WCH : (s + 1) * WCH, :]
                        )
                        wb = wbinp.tile([P, WCH, OB], BF16, name=f"wb{s}", tag=f"wb{s}")
                        # all binarize on DVE: gpsimd tensor ops measure ~25x
                        # slower than the cost model claims and gate the MMs
                        nc.vector.tensor_scalar(
                            wb[:], wr[:].bitcast(U8), 128.0, 0.5, IS_LT, SUB
                        )
                        subs.append(wb)
                    return subs

                def mm_block(ob, subs):
                    # zT orientation: psum tiles [128 out, 512 tok];
                    # stationary = bf16 +-0.5 weight plane column, shared
                    # by the 2 token-chunk streams (LDWEIGHTS deduped).
                    for oi in range(OB // P):
                        pms = [
                            psmp.tile([P, TH], F32, name="pm", tag="pm")
                            for _ in range(NTH)
                        ]
                        for k in range(KT):
                            wb = subs[k // WCH]
                            st = wb[:, k % WCH, ts(oi, P)]
                            xk = xt[k // XG]
                            for th in range(NTH):
                                nc.tensor.matmul(
                                    pms[th][:],
                                    st,
                                    xk[:, k % XG, ts(th, TH)],
                                    start=(k == 0),
                                    stop=(k == KT - 1),
                                )
                        zt = zstp.tile([P, T], F32, name="zt", tag="zt")
                        for th in range(NTH):
                            nc.scalar.copy(zt[:, ts(th, TH)], pms[th][:])
                        nc.sync.dma_start(zs_d.ap()[ts(ob * 2 + oi, P), :], zt[:])

                # x stream starts first (SP queue), w blocks 0/1 prep on the
                # ACT queue in parallel; early x groups are prioritized so the
                # matmul stream never starves while w competes for HBM
                nc.sync.dma_start(xt[0][:], x2_d.ap()[0])
                wb_cur = prep(0, 0)
                for g in range(1, 2):
                    nc.sync.dma_start(xt[g][:], x2_d.ap()[g])
                wb_next = prep(1, 1)
                for g in range(2, NXG):
                    nc.sync.dma_start(xt[g][:], x2_d.ap()[g])

                for ob in range(NOB):
                    mm_block(ob, wb_cur)
                    wb_cur = wb_next
                    wb_next = prep(ob + 2, ob) if ob + 2 < NOB else None
    n = _dedupe_ldweights(nc)
    assert 900 <= n <= NOB * (OB // P) * KT, n
    nc.compile()
    return nc


def _get_nc():
    global _cached_nc
    if _cached_nc is None:
        _cached_nc = _build_program()
    return _cached_nc


def _host_inputs(x, weight):
    """Per-core input dicts (layout/transport prep only):
    x2: bf16(2*x_shard.T) grouped [NXG, P, XG, T];
    wt: fp8(w.T) block-major [NOB, P, KT, OB] (sign-exact), replicated."""
    w8 = weight.T.astype(ml_dtypes.float8_e4m3)  # [K, OUT], sign preserved
    wt = np.ascontiguousarray(
        w8.reshape(KT, P, NOB, OB).transpose(2, 1, 0, 3)
    )
    x2 = (2.0 * x).astype(np.float32)
    in_maps = []
    for c in range(N_CORES):
        x2t = np.ascontiguousarray(
            x2[c * T : (c + 1) * T]
            .T.reshape(NXG, XG, P, T)
            .transpose(0, 2, 1, 3)
        ).astype(ml_dtypes.bfloat16)
        in_maps.append({"x2": x2t, "wt": wt})
    return in_maps


def kernel(x: np.ndarray, weight: np.ndarray, bias: np.ndarray) -> np.ndarray:
    x = np.ascontiguousarray(np.asarray(x, dtype=np.float32))
    weight = np.ascontiguousarray(np.asarray(weight, dtype=np.float32))
    bias = np.ascontiguousarray(np.asarray(bias, dtype=np.float32))
    assert x.shape == (N_TOK, K_IN) and weight.shape == (N_OUT, K_IN)

    nc = _get_nc()
    in_maps = _host_inputs(x, weight)
    res = bass_utils.run_bass_kernel_spmd(nc, in_maps, core_ids=list(range(N_CORES)))
    z = np.empty((N_TOK, N_OUT), dtype=np.float32)
    for c in range(N_CORES):
        np.copyto(z[c * T : (c + 1) * T], res.results[c]["zs"].T)
    if np.any(bias):
        z += bias[None, :]
    return z


# ---------------------------------------------------------------------------
# HW timing support (not used by the grading path; test.py calls this).
# The axon PJRT dispatch overhead (~57-110 ms, noisy) swamps a single kernel
# execution, so we measure on-device time with For_i-looped variants:
# slope of min wall time vs loop count. Dispatch noise is additive-positive,
# so the global min per L over all rounds is the robust estimator; three L
# values let us check linearity.
# ---------------------------------------------------------------------------


def _make_runner(nc, n_cores=N_CORES):
    import jax
    from jax.sharding import Mesh, PartitionSpec
    from jax.experimental.shard_map import shard_map
    from concourse import bass2jax

    bass2jax.install_neuronx_cc_hook()
    partition_name = nc.partition_id_tensor.name if nc.partition_id_tensor else None
    in_names, out_names, out_avals, zero_outs = [], [], [], []
    for alloc in nc.m.functions[0].allocations:
        if not isinstance(alloc, mybir.MemoryLocationSet):
            continue
        name = alloc.memorylocations[0].name
        if alloc.kind == "ExternalInput":
            if name != partition_name:
                in_names.append(name)
        elif alloc.kind == "ExternalOutput":
            out_names.append(name)
            out_avals.append(
                jax.core.ShapedArray(tuple(alloc.tensor_shape), mybir.dt.np(alloc.dtype))
            )
            zero_outs.append(
                np.zeros(tuple(alloc.tensor_shape), mybir.dt.np(alloc.dtype))
            )
    n_params, n_outs = len(in_names), len(out_avals)
    all_in_names = list(in_names) + list(out_names)
    if partition_name is not None:
        all_in_names.append(partition_name)

    def _body(*args):
        operands = list(args)
        if partition_name is not None:
            operands.append(bass2jax.partition_id_tensor())
        return tuple(
            bass2jax._bass_exec_p.bind(
                *operands,
                out_avals=tuple(out_avals),
                in_names=tuple(all_in_names),
                out_names=tuple(out_names),
                lowering_input_output_aliases=(),
                sim_require_finite=True,
                sim_require_nnan=True,
                nc=nc,
            )
        )

    donate = tuple(range(n_params, n_params + n_outs))
    devices = jax.devices()[:n_cores]
    mesh = Mesh(np.asarray(devices), ("core",))
    in_specs = (PartitionSpec("core"),) * (n_params + n_outs)
    out_specs = (PartitionSpec("core"),) * n_outs
    jitted = jax.jit(
        shard_map(_body, mesh=mesh, in_specs=in_specs, out_specs=out_specs,
                  check_rep=False),
        donate_argnums=donate,
        keep_unused=True,
    )
    return jitted, in_names, zero_outs


def _min_wall(jitted, ins, zero_outs_global, nrep):
    import time
    import jax

    best = float("inf")
    for _ in range(nrep):
        zo = [jax.device_put(z) for z in zero_outs_global]
        jax.block_until_ready(zo)
        t0 = time.perf_counter()
        outs = jitted(*ins, *zo)
        jax.block_until_ready(outs)
        best = min(best, time.perf_counter() - t0)
    return best


def measure_hw_time_ns(inputs, Ls=(1, 65, 129), nrep=6, rounds=6):
    import jax

    x = np.ascontiguousarray(np.asarray(inputs["x"], dtype=np.float32))
    weight = np.ascontiguousarray(np.asarray(inputs["weight"], dtype=np.float32))
    in_maps = _host_inputs(x, weight)

    runners = []
    for L in Ls:
        nc = _build_program(loop=L)
        jitted, in_names, zero_outs = _make_runner(nc)
        concat_in = [
            np.concatenate(
                [np.asarray(in_maps[c][name]) for c in range(N_CORES)], axis=0
            )
            for name in in_names
        ]
        ins = [jax.device_put(a) for a in concat_in]
        jax.block_until_ready(ins)
        zo_global = [np.concatenate([z] * N_CORES, axis=0) for z in zero_outs]
        # warmup
        outs = jitted(*ins, *[jax.device_put(z) for z in zo_global])
        jax.block_until_ready(outs)
        runners.append((jitted, ins, zo_global))

    t_min = [float("inf")] * len(Ls)
    for r in range(rounds):
        for i in range(len(Ls)):
            jitted, ins, zo = runners[i]
            t_min[i] = min(t_min[i], _min_wall(jitted, ins, zo, nrep))
        msg = " ".join(f"t({L})={t_min[i]*1e3:.2f}ms" for i, L in enumerate(Ls))
        pair = (t_min[-1] - t_min[0]) / (Ls[-1] - Ls[0]) * 1e9
        print(f"  timing round {r}: {msg} slope={pair:.0f}ns")
    # least-squares slope over the L points
    Lv = np.asarray(Ls, dtype=np.float64)
    tv = np.asarray(t_min, dtype=np.float64)
    slope = float(np.cov(Lv, tv, bias=True)[0, 1] / np.var(Lv))
    return slope * 1e9


# revision 22
# speedup vs baseline: 1.0014x; 1.0014x over previous
"""BinaryLinear TRN2 kernel: z = x @ sign(weight).T + bias.

x [8192, 4096] f32, weight [4096, 4096] f32, bias [4096] f32 (zeros).

Strategy (8 NeuronCores, SPMD, data-parallel over the 8192-token batch;
weight replicated; host does layout/transport prep only, all arithmetic
on device):

  - bf16 single-pass matmul. The PE moving feed is the limit: bf16
    streams 1 column/cycle (16384 MAC/cyc); fp8 DoubleRow doubles that
    but needs a two-pass hi/lo split for precision, landing at the same
    ~437 us/core floor (measured 444-454 us on HW for both). bf16
    single-pass hits the floor with half the instructions, no x
    quantization, FWL-eligible 128-col weight loads, ~1.6e-3 rel err.
  - Host ships bf16(2*x_shard.T) and fp8(w.T) (sign-exact transport,
    half the bytes) in BLOCK-MAJOR layouts so every DMA is long
    contiguous runs per partition (2-8 KB) - transposed-layout 512 B
    runs were the previous bottleneck (~90 GB/s effective; the 33.5 MB
    weight stream dominated the kernel).
  - Device binarize: bitcast the fp8 weight to uint8 and take the sign
    bit: (u8 is_lt 128) - 0.5 = sign(w)/2 with sign(-0)=-1 for
    rounded-up tiny negatives and sign(+0)=+1 - exactly matching the
    reference's sign(0)=+1. One DVE/Pool tensor_scalar per sub-chunk.
    z = (sign(w)/2) @ (2x) exactly.
  - PE runs ONLY matmuls: stationary [128,128] bf16 +-0.5 weights,
    moving 512 tokens, psum [128,512] (one bank), 32-deep accumulation,
    stationary shared across the 2 token chunks (redundant LDWEIGHTS
    deleted post-legalization - the PE array keeps its weights).
  - x streams on the SP DMA queue (prioritized at startup), w blocks on
    the ACT queue in 8-plane sub-chunks binarized ahead (2 blocks in
    flight); z written as bf16 zT, host transposes/upcasts on gather.
"""

import numpy as np
import ml_dtypes

import concourse.bacc as bacc
import concourse.bass as bass
import concourse.mybir as mybir
import concourse.tile as tile
from concourse import bass_utils
from concourse.bass import ts

P = 128
N_CORES = 8
N_TOK, K_IN, N_OUT = 8192, 4096, 4096
T = N_TOK // N_CORES  # 1024 tokens per core
KT = K_IN // P  # 32 k-planes
XG = 4  # x planes per DMA group
NXG = KT // XG  # 8 x tiles
OB = 256  # out-feature block width
NOB = N_OUT // OB  # 16 blocks
TH = 512  # token chunk per psum tile
NTH = T // TH  # 2
WCH = 8  # k-planes per w prep sub-chunk
NWCH = KT // WCH  # 4 sub-chunks per block

F32 = mybir.dt.float32
BF16 = mybir.dt.bfloat16
F8 = mybir.dt.float8e4
U8 = mybir.dt.uint8

_cached_nc = None


def _dedupe_ldweights(nc):
    """Remove InstLdweights whose weight AP is identical to the previous
    one with only InstMatmult instructions in between (the PE array still
    holds those weights). Waits on a removed load are hoisted onto the
    next PE instruction."""
    n_removed = 0
    for blk in nc.m.functions[0].blocks:
        insts = blk.instructions
        keep = []
        last_ld_key = None
        pending_waits = []
        for inst in insts:
            if inst.engine != mybir.EngineType.PE:
                keep.append(inst)
                continue
            if isinstance(inst, mybir.InstLdweights):
                key = str(inst.ins[0]) + f"|{inst.perf_mode}"
                if key == last_ld_key:
                    si = inst.sync_info
                    if si is not None:
                        pending_waits.extend(si.on_wait)
                        assert not si.on_update, "dedupe: LD carries updates"
                    n_removed += 1
                    continue
                last_ld_key = key
            elif not isinstance(inst, mybir.InstMatmult):
                last_ld_key = None
            if pending_waits:
                si = inst.sync_info
                if si is None:
                    inst.sync_info = mybir.SyncInfo(
                        on_wait=pending_waits, on_update=[]
                    )
                else:
                    si.on_wait = list(si.on_wait) + pending_waits
                pending_waits = []
            keep.append(inst)
        assert not pending_waits
        blk.instructions = keep
    return n_removed


def _build_program(loop: int = 0):
    """loop=0: plain kernel. loop=L>0: body wrapped in an on-device For_i
    (used for HW timing via the slope method)."""
    nc = bacc.Bacc("TRN2", target_bir_lowering=False, debug=False)
    # bf16(2*x) shard: [group, kp, plane-in-group, tok] (8 KB runs)
    x2_d = nc.dram_tensor("x2", [NXG, P, XG, T], BF16, kind="ExternalInput")
    # fp8(w.T), block-major: [block, kp, plane, out-in-block] (8 KB runs)
    w_d = nc.dram_tensor("wt", [NOB, P, KT, OB], F8, kind="ExternalInput")
    zs_d = nc.dram_tensor("zs", [N_OUT, T], F32, kind="ExternalOutput")

    import contextlib

    IS_LT = mybir.AluOpType.is_lt
    SUB = mybir.AluOpType.subtract

    with tile.TileContext(nc) as tc:
        with (
            tc.tile_pool(name="xt", bufs=1) as xtp,
            tc.tile_pool(name="wraw", bufs=8) as wrawp,
            tc.tile_pool(name="wbin", bufs=3) as wbinp,
            tc.tile_pool(name="zst", bufs=3) as zstp,
            tc.tile_pool(name="psm", bufs=6, space="PSUM") as psmp,
        ):
            # resident bf16 x tiles, one per XG-plane group
            xt = [
                xtp.tile([P, XG, T], BF16, name=f"xt{i}") for i in range(NXG)
            ]

            loop_cm = tc.For_i(0, loop, 1) if loop else contextlib.nullcontext()
            with loop_cm:
                # ---- Weight block prep (ACT hwdge queue): DMA fp8 wT
                # sub-chunks (contiguous 2 KB runs), binarize via the uint8
                # sign bit to +-0.5 bf16 on DVE/Pool ----
                def prep(ob, eng_off=0):
                    subs = []
                    for s in range(NWCH):
                        wr = wrawp.tile([P, WCH, OB], F8, name="wr", tag="wr")
                        nc.scalar.dma_start(
                            wr[:], w_d.ap()[ob, :, s * WCH : (s + 1) * WCH, :]
                        )
                        wb = wbinp.tile([P, WCH, OB], BF16, name=f"wb{s}", tag=f"wb{s}")
                        # all binarize on DVE: gpsimd tensor ops measure ~25x
                        # slower than the cost model claims and gate the MMs
                        nc.vector.tensor_scalar(
                            wb[:], wr[:].bitcast(U8), 128.0, 0.5, IS_LT, SUB
                        )
                        subs.append(wb)
                    return subs

                def mm_block(ob, subs):
                    # zT orientation: psum tiles [128 out, 512 tok];
                    # stationary = bf16 +-0.5 weight plane column, shared
                    # by the 2 token-chunk streams (LDWEIGHTS deduped).
                    for oi in range(OB // P):
                        pms = [
                            psmp.tile([P, TH], F32, name="pm", tag="pm")
                            for _ in range(NTH)
                        ]
                        for k in range(KT):
                            wb = subs[k // WCH]
                            st = wb[:, k % WCH, ts(oi, P)]
                            xk = xt[k // XG]
                            for th in range(NTH):
                                nc.tensor.matmul(
                                    pms[th][:],
                                    st,
                                    xk[:, k % XG, ts(th, TH)],
                                    start=(k == 0),
                                    stop=(k == KT - 1),
                                )
                        zt = zstp.tile([P, T], F32, name="zt", tag="zt")
                        for th in range(NTH):
                            nc.scalar.copy(zt[:, ts(th, TH)], pms[th][:])
                        # z goes out on the ACT queue: keeping SP x-only lets
                        # the next loop iteration's x stream start as soon as
                        # its WAR clears instead of queuing behind z stores
                        nc.scalar.dma_start(zs_d.ap()[ts(ob * 2 + oi, P), :], zt[:])

                # x stream starts first (SP queue), w blocks 0/1 prep on the
                # ACT queue in parallel; early x groups are prioritized so the
                # matmul stream never starves while w competes for HBM
                nc.sync.dma_start(xt[0][:], x2_d.ap()[0])
                wb_cur = prep(0, 0)
                for g in range(1, 2):
                    nc.sync.dma_start(xt[g][:], x2_d.ap()[g])
                wb_next = prep(1, 1)
                for g in range(2, NXG):
                    nc.sync.dma_start(xt[g][:], x2_d.ap()[g])

                for ob in range(NOB):
                    mm_block(ob, wb_cur)
                    wb_cur = wb_next
                    wb_next = prep(ob + 2, ob) if ob + 2 < NOB else None
    n = _dedupe_ldweights(nc)
    assert 900 <= n <= NOB * (OB // P) * KT, n
    nc.compile()
    return nc


def _get_nc():
    global _cached_nc
    if _cached_nc is None:
        _cached_nc = _build_program()
    return _cached_nc


def _host_inputs(x, weight):
    """Per-core input dicts (layout/transport prep only):
    x2: bf16(2*x_shard.T) grouped [NXG, P, XG, T];
    wt: fp8(w.T) block-major [NOB, P, KT, OB] (sign-exact), replicated."""
    w8 = weight.T.astype(ml_dtypes.float8_e4m3)  # [K, OUT], sign preserved
    wt = np.ascontiguousarray(
        w8.reshape(KT, P, NOB, OB).transpose(2, 1, 0, 3)
    )
    x2 = (2.0 * x).astype(np.float32)
    in_maps = []
    for c in range(N_CORES):
        x2t = np.ascontiguousarray(
            x2[c * T : (c + 1) * T]
            .T.reshape(NXG, XG, P, T)
            .transpose(0, 2, 1, 3)
        ).astype(ml_dtypes.bfloat16)
        in_maps.append({"x2": x2t, "wt": wt})
    return in_maps


def kernel(x: np.ndarray, weight: np.ndarray, bias: np.ndarray) -> np.ndarray:
    x = np.ascontiguousarray(np.asarray(x, dtype=np.float32))
    weight = np.ascontiguousarray(np.asarray(weight, dtype=np.float32))
    bias = np.ascontiguousarray(np.asarray(bias, dtype=np.float32))
    assert x.shape == (N_TOK, K_IN) and weight.shape == (N_OUT, K_IN)

    nc = _get_nc()
    in_maps = _host_inputs(x, weight)
    res = bass_utils.run_bass_kernel_spmd(nc, in_maps, core_ids=list(range(N_CORES)))
    z = np.empty((N_TOK, N_OUT), dtype=np.float32)
    for c in range(N_CORES):
        np.copyto(z[c * T : (c + 1) * T], res.results[c]["zs"].T)
    if np.any(bias):
        z += bias[None, :]
    return z


# ---------------------------------------------------------------------------
# HW timing support (not used by the grading path; test.py calls this).
# The axon PJRT dispatch overhead (~57-110 ms, noisy) swamps a single kernel
# execution, so we measure on-device time with For_i-looped variants:
# slope of min wall time vs loop count. Dispatch noise is additive-positive,
# so the global min per L over all rounds is the robust estimator; three L
# values let us check linearity.
# ---------------------------------------------------------------------------


def _make_runner(nc, n_cores=N_CORES):
    import jax
    from jax.sharding import Mesh, PartitionSpec
    from jax.experimental.shard_map import shard_map
    from concourse import bass2jax

    bass2jax.install_neuronx_cc_hook()
    partition_name = nc.partition_id_tensor.name if nc.partition_id_tensor else None
    in_names, out_names, out_avals, zero_outs = [], [], [], []
    for alloc in nc.m.functions[0].allocations:
        if not isinstance(alloc, mybir.MemoryLocationSet):
            continue
        name = alloc.memorylocations[0].name
        if alloc.kind == "ExternalInput":
            if name != partition_name:
                in_names.append(name)
        elif alloc.kind == "ExternalOutput":
            out_names.append(name)
            out_avals.append(
                jax.core.ShapedArray(tuple(alloc.tensor_shape), mybir.dt.np(alloc.dtype))
            )
            zero_outs.append(
                np.zeros(tuple(alloc.tensor_shape), mybir.dt.np(alloc.dtype))
            )
    n_params, n_outs = len(in_names), len(out_avals)
    all_in_names = list(in_names) + list(out_names)
    if partition_name is not None:
        all_in_names.append(partition_name)

    def _body(*args):
        operands = list(args)
        if partition_name is not None:
            operands.append(bass2jax.partition_id_tensor())
        return tuple(
            bass2jax._bass_exec_p.bind(
                *operands,
                out_avals=tuple(out_avals),
                in_names=tuple(all_in_names),
                out_names=tuple(out_names),
                lowering_input_output_aliases=(),
                sim_require_finite=True,
                sim_require_nnan=True,
                nc=nc,
            )
        )

    donate = tuple(range(n_params, n_params + n_outs))
    devices = jax.devices()[:n_cores]
    mesh = Mesh(np.asarray(devices), ("core",))
    in_specs = (PartitionSpec("core"),) * (n_params + n_outs)
    out_specs = (PartitionSpec("core"),) * n_outs
    jitted = jax.jit(
        shard_map(_body, mesh=mesh, in_specs=in_specs, out_specs=out_specs,
                  check_rep=False),
        donate_argnums=donate,
        keep_unused=True,
    )
    return jitted, in_names, zero_outs


def _min_wall(jitted, ins, zero_outs_global, nrep):
    import time
    import jax

    best = float("inf")
    for _ in range(nrep):
        zo = [jax.device_put(z) for z in zero_outs_global]
        jax.block_until_ready(zo)
        t0 = time.perf_counter()
        outs = jitted(*ins, *zo)
        jax.block_until_ready(outs)
        best = min(best, time.perf_counter() - t0)
    return best


def measure_hw_time_ns(inputs, Ls=(1, 65, 129), nrep=6, rounds=6):
    import jax

    x = np.ascontiguousarray(np.asarray(inputs["x"], dtype=np.float32))
    weight = np.ascontiguousarray(np.asarray(inputs["weight"], dtype=np.float32))
    in_maps = _host_inputs(x, weight)

    runners = []
    for L in Ls:
        nc = _build_program(loop=L)
        jitted, in_names, zero_outs = _make_runner(nc)
        concat_in = [
            np.concatenate(
                [np.asarray(in_maps[c][name]) for c in range(N_CORES)], axis=0
            )
            for name in in_names
        ]
        ins = [jax.device_put(a) for a in concat_in]
        jax.block_until_ready(ins)
        zo_global = [np.concatenate([z] * N_CORES, axis=0) for z in zero_outs]
        # warmup
        outs = jitted(*ins, *[jax.device_put(z) for z in zo_global])
        jax.block_until_ready(outs)
        runners.append((jitted, ins, zo_global))

    t_min = [float("inf")] * len(Ls)
    for r in range(rounds):
        for i in range(len(Ls)):
            jitted, ins, zo = runners[i]
            t_min[i] = min(t_min[i], _min_wall(jitted, ins, zo, nrep))
        msg = " ".join(f"t({L})={t_min[i]*1e3:.2f}ms" for i, L in enumerate(Ls))
        pair = (t_min[-1] - t_min[0]) / (Ls[-1] - Ls[0]) * 1e9
        print(f"  timing round {r}: {msg} slope={pair:.0f}ns")
    # least-squares slope over the L points
    Lv = np.asarray(Ls, dtype=np.float64)
    tv = np.asarray(t_min, dtype=np.float64)
    slope = float(np.cov(Lv, tv, bias=True)[0, 1] / np.var(Lv))
    return slope * 1e9


# revision 25
# speedup vs baseline: 1.0086x; 1.0072x over previous
"""BinaryLinear TRN2 kernel: z = x @ sign(weight).T + bias.

x [8192, 4096] f32, weight [4096, 4096] f32, bias [4096] f32 (zeros).

Strategy (8 NeuronCores, SPMD, data-parallel over the 8192-token batch;
weight replicated; host does layout/transport prep only, all arithmetic
on device):

  - bf16 single-pass matmul. The PE moving feed is the limit: bf16
    streams 1 column/cycle (16384 MAC/cyc); fp8 DoubleRow doubles that
    but needs a two-pass hi/lo split for precision, landing at the same
    ~437 us/core floor (measured 444-454 us on HW for both). bf16
    single-pass hits the floor with half the instructions, no x
    quantization, FWL-eligible 128-col weight loads, ~1.6e-3 rel err.
  - Host ships bf16(2*x_shard.T) and fp8(w.T) (sign-exact transport,
    half the bytes) in BLOCK-MAJOR layouts so every DMA is long
    contiguous runs per partition (2-8 KB) - transposed-layout 512 B
    runs were the previous bottleneck (~90 GB/s effective; the 33.5 MB
    weight stream dominated the kernel).
  - Device binarize: bitcast the fp8 weight to uint8 and take the sign
    bit: (u8 is_lt 128) - 0.5 = sign(w)/2 with sign(-0)=-1 for
    rounded-up tiny negatives and sign(+0)=+1 - exactly matching the
    reference's sign(0)=+1. One DVE/Pool tensor_scalar per sub-chunk.
    z = (sign(w)/2) @ (2x) exactly.
  - PE runs ONLY matmuls: stationary [128,128] bf16 +-0.5 weights,
    moving 512 tokens, psum [128,512] (one bank), 32-deep accumulation,
    stationary shared across the 2 token chunks (redundant LDWEIGHTS
    deleted post-legalization - the PE array keeps its weights).
  - x streams on the SP DMA queue (prioritized at startup), w blocks on
    the ACT queue in 8-plane sub-chunks binarized ahead (2 blocks in
    flight); z written as bf16 zT, host transposes/upcasts on gather.
"""

import numpy as np
import ml_dtypes

import concourse.bacc as bacc
import concourse.bass as bass
import concourse.mybir as mybir
import concourse.tile as tile
from concourse import bass_utils
from concourse.bass import ts

P = 128
N_CORES = 8
N_TOK, K_IN, N_OUT = 8192, 4096, 4096
T = N_TOK // N_CORES  # 1024 tokens per core
KT = K_IN // P  # 32 k-planes
XG = 4  # x planes per DMA group
NXG = KT // XG  # 8 x tiles
OB = 256  # out-feature block width
NOB = N_OUT // OB  # 16 blocks
TH = 512  # token chunk per psum tile
NTH = T // TH  # 2
WCH = 8  # k-planes per w prep sub-chunk
NWCH = KT // WCH  # 4 sub-chunks per block

F32 = mybir.dt.float32
BF16 = mybir.dt.bfloat16
F8 = mybir.dt.float8e4
U8 = mybir.dt.uint8

_cached_nc = None


def _dedupe_ldweights(nc):
    """Remove InstLdweights whose weight AP is identical to the previous
    one with only InstMatmult instructions in between (the PE array still
    holds those weights). Waits on a removed load are hoisted onto the
    next PE instruction."""
    n_removed = 0
    for blk in nc.m.functions[0].blocks:
        insts = blk.instructions
        keep = []
        last_ld_key = None
        pending_waits = []
        for inst in insts:
            if inst.engine != mybir.EngineType.PE:
                keep.append(inst)
                continue
            if isinstance(inst, mybir.InstLdweights):
                key = str(inst.ins[0]) + f"|{inst.perf_mode}"
                if key == last_ld_key:
                    si = inst.sync_info
                    if si is not None:
                        pending_waits.extend(si.on_wait)
                        assert not si.on_update, "dedupe: LD carries updates"
                    n_removed += 1
                    continue
                last_ld_key = key
            elif not isinstance(inst, mybir.InstMatmult):
                last_ld_key = None
            if pending_waits:
                si = inst.sync_info
                if si is None:
                    inst.sync_info = mybir.SyncInfo(
                        on_wait=pending_waits, on_update=[]
                    )
                else:
                    si.on_wait = list(si.on_wait) + pending_waits
                pending_waits = []
            keep.append(inst)
        assert not pending_waits
        blk.instructions = keep
    return n_removed


def _build_program(loop: int = 0):
    """loop=0: plain kernel. loop=L>0: body wrapped in an on-device For_i
    (used for HW timing via the slope method)."""
    nc = bacc.Bacc("TRN2", target_bir_lowering=False, debug=False)
    # bf16(2*x) shard: [group, kp, plane-in-group, tok] (8 KB runs)
    x2_d = nc.dram_tensor("x2", [NXG, P, XG, T], BF16, kind="ExternalInput")
    # fp8(w.T), block-major: [block, kp, plane, out-in-block] (8 KB runs)
    w_d = nc.dram_tensor("wt", [NOB, P, KT, OB], F8, kind="ExternalInput")
    zs_d = nc.dram_tensor("zs", [N_OUT, T], F32, kind="ExternalOutput")

    import contextlib

    IS_LT = mybir.AluOpType.is_lt
    SUB = mybir.AluOpType.subtract

    with tile.TileContext(nc) as tc:
        with (
            tc.tile_pool(name="xt", bufs=1) as xtp,
            tc.tile_pool(name="wraw", bufs=8) as wrawp,
            tc.tile_pool(name="wbin", bufs=3) as wbinp,
            tc.tile_pool(name="zst", bufs=3) as zstp,
            tc.tile_pool(name="psm", bufs=8, space="PSUM") as psmp,
        ):
            # resident bf16 x tiles, one per XG-plane group
            xt = [
                xtp.tile([P, XG, T], BF16, name=f"xt{i}") for i in range(NXG)
            ]

            loop_cm = tc.For_i(0, loop, 1) if loop else contextlib.nullcontext()
            with loop_cm:
                # ---- Weight block prep (ACT hwdge queue): DMA fp8 wT
                # sub-chunks (contiguous 2 KB runs), binarize via the uint8
                # sign bit to +-0.5 bf16 on DVE/Pool ----
                def prep(ob, eng_off=0):
                    subs = []
                    for s in range(NWCH):
                        wr = wrawp.tile([P, WCH, OB], F8, name="wr", tag="wr")
                        nc.scalar.dma_start(
                            wr[:], w_d.ap()[ob, :, s * WCH : (s + 1) * WCH, :]
                        )
                        wb = wbinp.tile([P, WCH, OB], BF16, name=f"wb{s}", tag=f"wb{s}")
                        # all binarize on DVE: gpsimd tensor ops measure ~25x
                        # slower than the cost model claims and gate the MMs
                        nc.vector.tensor_scalar(
                            wb[:], wr[:].bitcast(U8), 128.0, 0.5, IS_LT, SUB
                        )
                        subs.append(wb)
                    return subs

                def mm_block(ob, subs):
                    # zT orientation: psum tiles [128 out, 512 tok];
                    # stationary = bf16 +-0.5 weight plane column, shared
                    # by the 2 token-chunk streams (LDWEIGHTS deduped).
                    for oi in range(OB // P):
                        pms = [
                            psmp.tile([P, TH], F32, name="pm", tag="pm")
                            for _ in range(NTH)
                        ]
                        for k in range(KT):
                            wb = subs[k // WCH]
                            st = wb[:, k % WCH, ts(oi, P)]
                            xk = xt[k // XG]
                            for th in range(NTH):
                                nc.tensor.matmul(
                                    pms[th][:],
                                    st,
                                    xk[:, k % XG, ts(th, TH)],
                                    start=(k == 0),
                                    stop=(k == KT - 1),
                                )
                        zt = zstp.tile([P, T], F32, name="zt", tag="zt")
                        # evictions alternate DVE/ACT so the ACT queue's w-DMA
                        # triggers never stall behind a psum-gated copy
                        for th in range(NTH):
                            eng = nc.vector if (oi + th) % 2 == 0 else nc.scalar
                            (eng.tensor_copy if eng is nc.vector else eng.copy)(
                                zt[:, ts(th, TH)], pms[th][:]
                            )
                        # z goes out on the ACT queue: keeping SP x-only lets
                        # the next loop iteration's x stream start as soon as
                        # its WAR clears instead of queuing behind z stores
                        nc.scalar.dma_start(zs_d.ap()[ts(ob * 2 + oi, P), :], zt[:])

                # x stream starts first (SP queue), w blocks 0/1 prep on the
                # ACT queue in parallel; early x groups are prioritized so the
                # matmul stream never starves while w competes for HBM
                nc.sync.dma_start(xt[0][:], x2_d.ap()[0])
                wb_cur = prep(0, 0)
                for g in range(1, 2):
                    nc.sync.dma_start(xt[g][:], x2_d.ap()[g])
                wb_next = prep(1, 1)
                for g in range(2, NXG):
                    nc.sync.dma_start(xt[g][:], x2_d.ap()[g])

                for ob in range(NOB):
                    mm_block(ob, wb_cur)
                    wb_cur = wb_next
                    wb_next = prep(ob + 2, ob) if ob + 2 < NOB else None
    n = _dedupe_ldweights(nc)
    assert 900 <= n <= NOB * (OB // P) * KT, n
    nc.compile()
    return nc


def _get_nc():
    global _cached_nc
    if _cached_nc is None:
        _cached_nc = _build_program()
    return _cached_nc


def _host_inputs(x, weight):
    """Per-core input dicts (layout/transport prep only):
    x2: bf16(2*x_shard.T) grouped [NXG, P, XG, T];
    wt: fp8(w.T) block-major [NOB, P, KT, OB] (sign-exact), replicated."""
    w8 = weight.T.astype(ml_dtypes.float8_e4m3)  # [K, OUT], sign preserved
    wt = np.ascontiguousarray(
        w8.reshape(KT, P, NOB, OB).transpose(2, 1, 0, 3)
    )
    x2 = (2.0 * x).astype(np.float32)
    in_maps = []
    for c in range(N_CORES):
        x2t = np.ascontiguousarray(
            x2[c * T : (c + 1) * T]
            .T.reshape(NXG, XG, P, T)
            .transpose(0, 2, 1, 3)
        ).astype(ml_dtypes.bfloat16)
        in_maps.append({"x2": x2t, "wt": wt})
    return in_maps


def kernel(x: np.ndarray, weight: np.ndarray, bias: np.ndarray) -> np.ndarray:
    x = np.ascontiguousarray(np.asarray(x, dtype=np.float32))
    weight = np.ascontiguousarray(np.asarray(weight, dtype=np.float32))
    bias = np.ascontiguousarray(np.asarray(bias, dtype=np.float32))
    assert x.shape == (N_TOK, K_IN) and weight.shape == (N_OUT, K_IN)

    nc = _get_nc()
    in_maps = _host_inputs(x, weight)
    res = bass_utils.run_bass_kernel_spmd(nc, in_maps, core_ids=list(range(N_CORES)))
    z = np.empty((N_TOK, N_OUT), dtype=np.float32)
    for c in range(N_CORES):
        np.copyto(z[c * T : (c + 1) * T], res.results[c]["zs"].T)
    if np.any(bias):
        z += bias[None, :]
    return z


# ---------------------------------------------------------------------------
# HW timing support (not used by the grading path; test.py calls this).
# The axon PJRT dispatch overhead (~57-110 ms, noisy) swamps a single kernel
# execution, so we measure on-device time with For_i-looped variants:
# slope of min wall time vs loop count. Dispatch noise is additive-positive,
# so the global min per L over all rounds is the robust estimator; three L
# values let us check linearity.
# ---------------------------------------------------------------------------


def _make_runner(nc, n_cores=N_CORES):
    import jax
    from jax.sharding import Mesh, PartitionSpec
    from jax.experimental.shard_map import shard_map
    from concourse import bass2jax

    bass2jax.install_neuronx_cc_hook()
    partition_name = nc.partition_id_tensor.name if nc.partition_id_tensor else None
    in_names, out_names, out_avals, zero_outs = [], [], [], []
    for alloc in nc.m.functions[0].allocations:
        if not isinstance(alloc, mybir.MemoryLocationSet):
            continue
        name = alloc.memorylocations[0].name
        if alloc.kind == "ExternalInput":
            if name != partition_name:
                in_names.append(name)
        elif alloc.kind == "ExternalOutput":
            out_names.append(name)
            out_avals.append(
                jax.core.ShapedArray(tuple(alloc.tensor_shape), mybir.dt.np(alloc.dtype))
            )
            zero_outs.append(
                np.zeros(tuple(alloc.tensor_shape), mybir.dt.np(alloc.dtype))
            )
    n_params, n_outs = len(in_names), len(out_avals)
    all_in_names = list(in_names) + list(out_names)
    if partition_name is not None:
        all_in_names.append(partition_name)

    def _body(*args):
        operands = list(args)
        if partition_name is not None:
            operands.append(bass2jax.partition_id_tensor())
        return tuple(
            bass2jax._bass_exec_p.bind(
                *operands,
                out_avals=tuple(out_avals),
                in_names=tuple(all_in_names),
                out_names=tuple(out_names),
                lowering_input_output_aliases=(),
                sim_require_finite=True,
                sim_require_nnan=True,
                nc=nc,
            )
        )

    donate = tuple(range(n_params, n_params + n_outs))
    devices = jax.devices()[:n_cores]
    mesh = Mesh(np.asarray(devices), ("core",))
    in_specs = (PartitionSpec("core"),) * (n_params + n_outs)
    out_specs = (PartitionSpec("core"),) * n_outs
    jitted = jax.jit(
        shard_map(_body, mesh=mesh, in_specs=in_specs, out_specs=out_specs,
                  check_rep=False),
        donate_argnums=donate,
        keep_unused=True,
    )
    return jitted, in_names, zero_outs


def _min_wall(jitted, ins, zero_outs_global, nrep):
    import time
    import jax

    best = float("inf")
    for _ in range(nrep):
        zo = [jax.device_put(z) for z in zero_outs_global]
        jax.block_until_ready(zo)
        t0 = time.perf_counter()
        outs = jitted(*ins, *zo)
        jax.block_until_ready(outs)
        best = min(best, time.perf_counter() - t0)
    return best


def measure_hw_time_ns(inputs, Ls=(1, 129), nrep=5, rounds=5):
    import jax

    x = np.ascontiguousarray(np.asarray(inputs["x"], dtype=np.float32))
    weight = np.ascontiguousarray(np.asarray(inputs["weight"], dtype=np.float32))
    in_maps = _host_inputs(x, weight)

    runners = []
    for L in Ls:
        nc = _build_program(loop=L)
        jitted, in_names, zero_outs = _make_runner(nc)
        concat_in = [
            np.concatenate(
                [np.asarray(in_maps[c][name]) for c in range(N_CORES)], axis=0
            )
            for name in in_names
        ]
        ins = [jax.device_put(a) for a in concat_in]
        jax.block_until_ready(ins)
        zo_global = [np.concatenate([z] * N_CORES, axis=0) for z in zero_outs]
        # warmup
        outs = jitted(*ins, *[jax.device_put(z) for z in zo_global])
        jax.block_until_ready(outs)
        runners.append((jitted, ins, zo_global))

    t_min = [float("inf")] * len(Ls)
    for r in range(rounds):
        for i in range(len(Ls)):
            jitted, ins, zo = runners[i]
            t_min[i] = min(t_min[i], _min_wall(jitted, ins, zo, nrep))
        msg = " ".join(f"t({L})={t_min[i]*1e3:.2f}ms" for i, L in enumerate(Ls))
        pair = (t_min[-1] - t_min[0]) / (Ls[-1] - Ls[0]) * 1e9
        print(f"  timing round {r}: {msg} slope={pair:.0f}ns")
    # least-squares slope over the L points
    Lv = np.asarray(Ls, dtype=np.float64)
    tv = np.asarray(t_min, dtype=np.float64)
    slope = float(np.cov(Lv, tv, bias=True)[0, 1] / np.var(Lv))
    return slope * 1e9
